# revision 31
# baseline (speedup 1.0000x reference)
"""Causal multi-head attention (B=4, S=2048, HID=1024, 16 heads x 64) with RoPE
on 8 TRN2 NeuronCores.

Sharding: core c -> batch b = c//2, head-group hg = c%2 (8 heads each).

Schedule: projections are split into 4 column-waves; wave j+1's matmul chains
(and the previous chunk's o_proj chains) are emitted as fine-grained FILLER
instructions inside attention chunk j's inner loop, with the score->exp->ctx
pipeline software-skewed by 2 tiles. This keeps the Tensor engine queue free
of head-of-line stalls on the Scalar (exp) engine, holding the PE busy so it
stays in its fast p-state.

Diagonal score tiles only compute/exp/accumulate the causally visible column
range; the on-diagonal 128-col block is masked in-place with one triangular
0/1 multiply on Vector (q-chunk 0 keeps the simpler full-width masked path).

Outputs are fp16 partial o_proj results DMA'd straight to DRAM; the pairwise
head-group reduction happens on the host during unsharding (no collective).

All matmuls run in fp16 (fp32 PSUM accumulation).
"""
import os as _os
import numpy as np
from collections import deque
from contextlib import ExitStack

import concourse.bass as bass
import concourse.tile as tile
import concourse.mybir as mybir
from concourse import bacc
from concourse.alu_op_type import AluOpType
from concourse.bass_utils import run_bass_kernel_spmd

F32 = mybir.dt.float32
F16 = mybir.dt.float16
BF16 = mybir.dt.bfloat16
MM_DT = BF16 if _os.environ.get("KMM", "f16") == "bf16" else F16
AF = mybir.ActivationFunctionType
Alu = AluOpType

B, S, HID = 4, 2048, 1024
NH, HD = 16, 64
SCALE = 1.0 / np.sqrt(HD)
ROPE_BASE = 10000.0
NCORES = 8
HPC = 8          # heads per core
JC = 512         # head dims per core
NJ = 4           # q chunks of 512
NT = 16          # kk tiles of 128
NSC = 4          # s chunks of 512 for projections
NHC = 8          # hid chunks of 128 (contraction)

_PROGRAM = None


def build():
    nc = bacc.Bacc("TRN2", target_bir_lowering=False, debug=False)

    hsT_d = nc.declare_dram_parameter("hsT", [HID, S], MM_DT, isOutput=False)
    wq_d = nc.declare_dram_parameter("wqT", [HID, JC], MM_DT, isOutput=False)
    wk_d = nc.declare_dram_parameter("wkT", [HID, JC], MM_DT, isOutput=False)
    wv_d = nc.declare_dram_parameter("wvT", [HID, JC], MM_DT, isOutput=False)
    wo_d = nc.declare_dram_parameter("woT", [JC, HID], MM_DT, isOutput=False)
    cos_d = nc.declare_dram_parameter("cosT2", [64, S], MM_DT, isOutput=False)
    sel2_d = nc.declare_dram_parameter("sel2", [128, 128], MM_DT, isOutput=False)
    sin_d = nc.declare_dram_parameter("sinT2", [64, S], MM_DT, isOutput=False)
    out_d = nc.declare_dram_parameter("out", [S, HID], F16, isOutput=True)

    rdram = nc.dram_tensor("rdram", [HPC, S], F32)

    with ExitStack() as ctx:
        tc = ctx.enter_context(tile.TileContext(nc, num_cores=NCORES))
        consts = ctx.enter_context(tc.tile_pool(name="consts", bufs=1))
        rt = ctx.enter_context(tc.tile_pool(name="rt", bufs=6))
        ptp = ctx.enter_context(tc.tile_pool(name="ptp", bufs=6))
        misc = ctx.enter_context(tc.tile_pool(name="misc", bufs=2))
        outp = ctx.enter_context(tc.tile_pool(name="outp", bufs=5))
        psum = ctx.enter_context(tc.tile_pool(name="psum", bufs=2, space="PSUM"))

        # ---- load constants, prioritized by first use ----
        hsT = consts.tile([128, NHC, S], MM_DT, tag="hsT")
        wsb = {}
        for name, d in (("wq", wq_d), ("wk", wk_d), ("wv", wv_d)):
            wsb[name] = consts.tile([128, NHC, JC], MM_DT, tag=name, name=f"w_{name}")
        for hc in range(NHC):
            nc.sync.dma_start(out=wsb["wq"][:, hc, :], in_=wq_d[hc * 128:(hc + 1) * 128, :])
            nc.sync.dma_start(out=hsT[:, hc, 0:512], in_=hsT_d[hc * 128:(hc + 1) * 128, 0:512])
        for hc in range(NHC):
            nc.sync.dma_start(out=wsb["wk"][:, hc, :], in_=wk_d[hc * 128:(hc + 1) * 128, :])
        cos2 = consts.tile([128, S], MM_DT, tag="cos2")
        sin2 = consts.tile([128, S], MM_DT, tag="sin2")
        nc.sync.dma_start(out=cos2[0:64, :], in_=cos_d[:])
        nc.sync.dma_start(out=sin2[0:64, :], in_=sin_d[:])
        nc.vector.tensor_copy(out=cos2[64:128, :], in_=cos2[0:64, :])
        nc.vector.tensor_copy(out=sin2[64:128, :], in_=sin2[0:64, :])
        for hc in range(NHC):
            nc.sync.dma_start(out=wsb["wv"][:, hc, :], in_=wv_d[hc * 128:(hc + 1) * 128, :])
        for w in range(1, 4):
            for hc in range(NHC):
                nc.sync.dma_start(
                    out=hsT[:, hc, 512 * w:512 * (w + 1)],
                    in_=hsT_d[hc * 128:(hc + 1) * 128, 512 * w:512 * (w + 1)],
                )
        wo = consts.tile([128, 4, HID], MM_DT, tag="wo")
        nc.sync.dma_start(out=wo[:], in_=wo_d[:].rearrange("(c p) j -> p c j", p=128))

        # ---- causal masks: full-width per-offset (chunk 0) + triangular block ----
        mask2 = [consts.tile([128, 2, 512], MM_DT, tag=f"mask{i}", name=f"mask{i}")
                 for i in range(4)]
        for i in range(4):
            nc.vector.memset(mask2[i][:], 1.0)
            for hl in range(2):
                nc.gpsimd.affine_select(
                    out=mask2[i][:, hl, :], in_=mask2[i][:, hl, :],
                    pattern=[[1, 512]], compare_op=Alu.is_ge,
                    fill=0.0, base=-128 * i, channel_multiplier=-1,
                )
        sel2 = consts.tile([128, 128], MM_DT, tag="sel2")
        nc.sync.dma_start(out=sel2[:], in_=sel2_d[:])
        tri2 = consts.tile([128, 2, 128], MM_DT, tag="tri2")
        nc.vector.memset(tri2[:], 1.0)
        for hl in range(2):
            nc.gpsimd.affine_select(
                out=tri2[:, hl, :], in_=tri2[:, hl, :],
                pattern=[[1, 128]], compare_op=Alu.is_ge,
                fill=0.0, base=0, channel_multiplier=-1,
            )

        qrope = [consts.tile([128, S], MM_DT, tag=f"qrope{i}", name=f"qrope{i}")
                 for i in range(4)]
        krope = [consts.tile([128, S], MM_DT, tag=f"krope{i}", name=f"krope{i}")
                 for i in range(4)]
        v_sb = consts.tile([128, NT, HPC, HD + 2], MM_DT, tag="v_sb")
        nc.vector.memset(v_sb[:, :, :, HD:HD + 2], 1.0)
        ctx_sb = [consts.tile([128, S], MM_DT, tag=f"ctx{i}", name=f"ctx_sb{i}")
                  for i in range(4)]

        # ---- filler machinery: fine-grained PE work queued into attention ----
        fillers = deque()

        def emit_fillers(n):
            for _ in range(n):
                if not fillers:
                    return
                fillers.popleft()()

        def drain_fillers():
            while fillers:
                fillers.popleft()()

        # ---- projection chains (emitted via fillers for waves >= 1) ----
        def qk_chain_items(sc, wname, hp):
            ssl = slice(sc * 512, (sc + 1) * 512)
            jcol = hp * 128
            dest = qrope if wname == "wq" else krope
            state = {}

            def mk_mm(hc):
                def f():
                    if hc == 0:
                        state["ps"] = psum.tile([128, 512], F32, tag="mm",
                                                name=f"qkps_{wname}{hp}_{sc}")
                    nc.tensor.matmul(
                        out=state["ps"][:],
                        lhsT=wsb[wname][:, hc, jcol:jcol + 128],
                        rhs=hsT[:, hc, ssl],
                        start=(hc == 0), stop=(hc == NHC - 1),
                    )
                return f

            def fin():
                raw_sb = misc.tile([128, 512], MM_DT, tag="qraw", bufs=4,
                                   name=f"raw_{wname}{hp}_{sc}")
                nc.scalar.copy(out=raw_sb[:], in_=state["ps"][:])
                rot_sb = misc.tile([128, 512], MM_DT, tag="qrot", bufs=4,
                                   name=f"rot_{wname}{hp}_{sc}")
                for hl in range(2):
                    b0 = 64 * hl
                    nc.scalar.dma_start(out=rot_sb[b0:b0 + 32, :],
                                        in_=raw_sb[b0 + 1:b0 + 64:2, :])
                    nc.scalar.dma_start(out=rot_sb[b0 + 32:b0 + 64, :],
                                        in_=raw_sb[b0:b0 + 63:2, :])
                t1 = rt.tile([128, 512], MM_DT, tag="rt")
                t2 = rt.tile([128, 512], MM_DT, tag="rt")
                nc.vector.tensor_tensor(out=t1[:], in0=raw_sb[:], in1=cos2[:, ssl], op=Alu.mult)
                nc.vector.tensor_tensor(out=t2[:], in0=rot_sb[:], in1=sin2[:, ssl], op=Alu.mult)
                nc.vector.tensor_add(out=dest[hp][:, ssl], in0=t1[:], in1=t2[:])

            return [mk_mm(hc) for hc in range(NHC)] + [fin]

        def v_chain_items(st):
            state = {}

            def mk_mm(hc):
                def f():
                    if hc == 0:
                        state["ps"] = psum.tile([128, JC], F32, tag="mm",
                                                name=f"vps_{st}")
                    nc.tensor.matmul(
                        out=state["ps"][:],
                        lhsT=hsT[:, hc, st * 128:(st + 1) * 128],
                        rhs=wsb["wv"][:, hc, :],
                        start=(hc == 0), stop=(hc == NHC - 1),
                    )
                return f

            def fin():
                nc.vector.tensor_copy(
                    out=v_sb[:, st, :, 0:HD],
                    in_=state["ps"][:].rearrange("p (h d) -> p h d", h=HPC),
                )

            return [mk_mm(hc) for hc in range(NHC)] + [fin]

        def proj_wave_items(sc):
            items = []
            for wname in ("wq", "wk"):
                for hp in range(4):
                    items.extend(qk_chain_items(sc, wname, hp))
            for st in range(4 * sc, 4 * sc + 4):
                items.extend(v_chain_items(st))
            return items

        # ---- o_proj chains for chunk j (queued as fillers into attn(j+1)) ----
        def oproj_chain_items(st, jc2):
            ssl2 = slice(st * 128, (st + 1) * 128)
            osl = slice(jc2 * 512, (jc2 + 1) * 512)
            state = {}

            def mk_mm(kc):
                def f():
                    if kc == 0:
                        state["ps"] = psum.tile([128, 512], F32, tag="mm",
                                                name=f"ops_{st}_{jc2}")
                    nc.tensor.matmul(
                        out=state["ps"][:],
                        lhsT=ctx_sb[kc][:, ssl2],
                        rhs=wo[:, kc, osl],
                        start=(kc == 0), stop=(kc == 3),
                    )
                return f

            def fin():
                o_sb = outp.tile([128, 512], F16, tag="osb")
                nc.vector.tensor_copy(out=o_sb[:], in_=state["ps"][:])
                nc.sync.dma_start(out=out_d[ssl2, osl], in_=o_sb[:])

            return [mk_mm(kc) for kc in range(4)] + [fin]

        bc_tiles = {}

        def norm_one(j, hp):
            qsl = slice(j * 512, (j + 1) * 512)
            bc = bc_tiles[j][hp]
            nc.vector.reciprocal_approx_fast(out=bc[:], in_=bc[:])
            nc.vector.tensor_tensor(
                out=ctx_sb[hp][:, qsl], in0=ctx_sb[hp][:, qsl], in1=bc[:], op=Alu.mult,
            )

        def norm_block(j):
            for hp in range(4):
                norm_one(j, hp)

        def oproj_items(j):
            items = []
            for st in range(4 * j, 4 * j + 4):
                for jc2 in range(2):
                    items.extend(oproj_chain_items(st, jc2))
            return items

        # ---- attention chunk j: skew-2 pipeline with fillers ----
        def attn_block(j):
            qsl = slice(j * 512, (j + 1) * 512)
            nt = 4 * j + 4
            if j >= 1:
                # diagonal tiles first (i=0 is full width -> clean psum start),
                # then the full-width off-diagonal tiles; last one carries stop.
                order = [4 * j + i for i in range(4)] + list(range(4 * j))
            else:
                order = list(range(nt))
            for hp in range(4):
                ctx_ps = [psum.tile([HD + 2, 512], F32, tag="ctx", name=f"ctx_ps{_i}")
                          for _i in range(2)]
                pts = {}

                def s_part(slot):
                    t = order[slot]
                    diag_i = t - 4 * j
                    shrink = j >= 1 and diag_i >= 0
                    c0 = 128 * diag_i if shrink else 0
                    sc_ps = psum.tile([128, 2, 512], F32, tag="sc")
                    ksl = slice(t * 128, (t + 1) * 128)
                    for hl in range(2):
                        pr = slice(64 * hl, 64 * hl + 64)
                        nc.tensor.matmul(
                            out=sc_ps[:, hl, c0:512],
                            lhsT=krope[hp][pr, ksl],
                            rhs=qrope[hp][pr, 512 * j + c0:512 * (j + 1)],
                            start=True, stop=True,
                        )
                    pt = ptp.tile([128, 2, 512], MM_DT, tag="pt")
                    nc.scalar.activation(out=pt[:, :, c0:512], in_=sc_ps[:, :, c0:512],
                                         func=AF.Exp, scale=float(SCALE))
                    if diag_i >= 0:
                        if shrink:
                            # in-place triangular mask on the on-diagonal block
                            blk = slice(c0, c0 + 128)
                            nc.vector.tensor_tensor(
                                out=pt[:, :, blk], in0=pt[:, :, blk], in1=tri2[:],
                                op=Alu.mult,
                            )
                        else:
                            ptm = ptp.tile([128, 2, 512], MM_DT, tag="ptm", bufs=3)
                            nc.vector.tensor_tensor(
                                out=ptm[:], in0=pt[:], in1=mask2[diag_i][:], op=Alu.mult,
                            )
                            pt = ptm
                    pts[slot] = (pt, c0)

                def c_part(slot):
                    t = order[slot]
                    pt, c0 = pts.pop(slot)
                    for hl in range(2):
                        nc.tensor.matmul(
                            out=ctx_ps[hl][:, c0:512],
                            lhsT=v_sb[:, t, 2 * hp + hl, :],
                            rhs=pt[:, hl, c0:512],
                            start=(slot == 0), stop=(slot == nt - 1),
                        )

                for slot in range(nt + 2):
                    if slot < nt:
                        s_part(slot)
                    emit_fillers(1)
                    if slot >= 2:
                        c_part(slot - 2)
                    emit_fillers(1)
                    if slot == 6 and j == 3 and hp >= 1:
                        norm_one(3, hp - 1)
                emit_fillers(2)

                last = j == 3 and hp == 3
                for hl in range(2):
                    pr = slice(64 * hl, 64 * hl + 64)
                    srow = misc.tile([128, 512], F32, tag="srow", bufs=4)
                    nc.vector.tensor_copy(out=srow[64:65, :], in_=ctx_ps[hl][HD:HD + 1, :])
                    nc.gpsimd.dma_start(
                        out=rdram[2 * hp + hl:2 * hp + hl + 1, qsl],
                        in_=srow[64:65, :],
                    )
                    if last:
                        nc.scalar.copy(out=ctx_sb[hp][pr, qsl], in_=ctx_ps[hl][0:HD, :])
                    else:
                        nc.vector.tensor_copy(out=ctx_sb[hp][pr, qsl], in_=ctx_ps[hl][0:HD, :])
                bc = misc.tile([128, 512], F32, tag="bc", bufs=5, name=f"bc{j}_{hp}")
                if last:
                    for q4 in range(2):
                        for hl in range(2):
                            p0 = 64 * hl + 32 * q4
                            nc.sync.dma_start(
                                out=bc[p0:p0 + 32, :],
                                in_=rdram[2 * hp + hl:2 * hp + hl + 1, qsl].partition_broadcast(32),
                            )
                else:
                    nc.sync.dma_start(
                        out=bc[0:64, :],
                        in_=rdram[2 * hp:2 * hp + 1, qsl].partition_broadcast(64),
                    )
                    nc.sync.dma_start(
                        out=bc[64:128, :],
                        in_=rdram[2 * hp + 1:2 * hp + 2, qsl].partition_broadcast(64),
                    )
                bc_tiles.setdefault(j, []).append(bc)
                if j == 3 and hp == 3:
                    norm_one(3, 3)

        # ---- schedule ----
        # wave 0 projections as a straight block (DMA-limited startup)
        for it in proj_wave_items(0):
            it()

        fillers.extend(proj_wave_items(1))
        attn_block(0)
        drain_fillers()

        norm_block(0)
        fillers.extend(proj_wave_items(2))
        attn_block(1)
        drain_fillers()

        norm_block(1)
        fillers.extend(proj_wave_items(3))
        fillers.extend(oproj_items(0))
        attn_block(2)
        drain_fillers()

        norm_block(2)
        fillers.extend(oproj_items(1))
        fillers.extend(oproj_items(2))
        attn_block(3)
        drain_fillers()

        for it in oproj_items(3):
            it()

    nc.finalize()
    return nc


def _rope_tables():
    inv_freq = (1.0 / (ROPE_BASE ** (np.arange(0, HD, 2, dtype=np.float32) / np.float32(HD)))).astype(np.float32)
    t = np.arange(S, dtype=np.float32)
    freqs = np.outer(t, inv_freq).astype(np.float32)          # [S, 32]
    emb = np.concatenate([freqs, freqs], axis=-1)             # [S, 64]
    return np.cos(emb).astype(np.float32), np.sin(emb).astype(np.float32)


def _rot_weights(W):
    """Rows of Wr give rotated(x) = cat(-x2, x1) of x = W @ h per 64-dim head."""
    Wr = np.empty_like(W)
    for h in range(NH):
        b = h * HD
        Wr[b:b + 32] = -W[b + 1:b + HD:2]
        Wr[b + 32:b + HD] = W[b:b + HD:2]
    return Wr


def prepare_in_maps(hidden_states, Wq, Wk, Wv, Wo):
    sel2 = np.zeros((128, 128), dtype=np.float32)
    sel2[64, 0:64] = 1.0
    sel2[65, 64:128] = 1.0
    cos, sin = _rope_tables()                                  # [S, 64]
    cos2 = np.ascontiguousarray(cos.T)                         # [64, S]
    sin2 = np.ascontiguousarray(sin.T)
    # sign of the rotation (-x2 for d<32) folded into the sin table
    sin2[0:32] *= -1.0
    if MM_DT == F16:
        f16 = np.float16
    else:
        import ml_dtypes
        f16 = ml_dtypes.bfloat16
    in_maps = []
    for c in range(NCORES):
        b, hg = c // 2, c % 2
        sl = slice(JC * hg, JC * (hg + 1))
        m = {
            "hsT": np.ascontiguousarray(hidden_states[b].T).astype(f16),
            "wqT": np.ascontiguousarray(Wq[sl].T).astype(f16),
            "wkT": np.ascontiguousarray(Wk[sl].T).astype(f16),
            "wvT": np.ascontiguousarray(Wv[sl].T).astype(f16),
            "woT": np.ascontiguousarray(Wo[:, sl].T).astype(f16),
            "sel2": sel2.astype(f16),
            "cosT2": cos2.astype(f16),
            "sinT2": sin2.astype(f16),
        }
        in_maps.append(m)
    return in_maps


def run(inputs, trace=False, tmpdir=None):
    global _PROGRAM
    if _PROGRAM is None:
        _PROGRAM = build()
    nc = _PROGRAM
    in_maps = prepare_in_maps(
        np.asarray(inputs["hidden_states"], dtype=np.float32),
        np.asarray(inputs["Wq"], dtype=np.float32),
        np.asarray(inputs["Wk"], dtype=np.float32),
        np.asarray(inputs["Wv"], dtype=np.float32),
        np.asarray(inputs["Wo"], dtype=np.float32),
    )
    res = run_bass_kernel_spmd(nc, in_maps, list(range(NCORES)), trace=trace, tmpdir=tmpdir)
    out = np.empty((B, S, HID), dtype=np.float32)
    for b in range(B):
        lo = res.results[2 * b]["out"].astype(np.float32)
        hi = res.results[2 * b + 1]["out"].astype(np.float32)
        out[b] = lo + hi
    return out, res


def kernel(**inputs):
    out, _ = run(inputs)
    return out


# revision 32
# speedup vs baseline: 1.0197x; 1.0197x over previous
"""Causal multi-head attention (B=4, S=2048, HID=1024, 16 heads x 64) with RoPE
on 8 TRN2 NeuronCores.

Sharding: core c -> batch b = c//2, head-group hg = c%2 (8 heads each).

Schedule: projections are split into 4 column-waves; wave j+1's matmul chains
(and the previous chunk's o_proj chains) are emitted as fine-grained FILLER
instructions inside attention chunk j's inner loop, with the score->exp->ctx
pipeline software-skewed by 2 tiles. This keeps the Tensor engine queue free
of head-of-line stalls on the Scalar (exp) engine, holding the PE busy so it
stays in its fast p-state.

Diagonal score tiles only compute/exp/accumulate the causally visible column
range; the on-diagonal 128-col block is masked in-place with one triangular
0/1 multiply on Vector (q-chunk 0 keeps the simpler full-width masked path).

Outputs are fp16 partial o_proj results DMA'd straight to DRAM; the pairwise
head-group reduction happens on the host during unsharding (no collective).

All matmuls run in fp16 (fp32 PSUM accumulation).
"""
import os as _os
import numpy as np
from collections import deque
from contextlib import ExitStack

import concourse.bass as bass
import concourse.tile as tile
import concourse.mybir as mybir
from concourse import bacc
from concourse.alu_op_type import AluOpType
from concourse.bass_utils import run_bass_kernel_spmd

F32 = mybir.dt.float32
F16 = mybir.dt.float16
BF16 = mybir.dt.bfloat16
MM_DT = BF16 if _os.environ.get("KMM", "f16") == "bf16" else F16
AF = mybir.ActivationFunctionType
Alu = AluOpType

B, S, HID = 4, 2048, 1024
NH, HD = 16, 64
SCALE = 1.0 / np.sqrt(HD)
ROPE_BASE = 10000.0
NCORES = 8
HPC = 8          # heads per core
JC = 512         # head dims per core
NJ = 4           # q chunks of 512
NT = 16          # kk tiles of 128
NSC = 4          # s chunks of 512 for projections
NHC = 8          # hid chunks of 128 (contraction)

_PROGRAM = None


def build():
    nc = bacc.Bacc("TRN2", target_bir_lowering=False, debug=False)

    hsT_d = nc.declare_dram_parameter("hsT", [HID, S], MM_DT, isOutput=False)
    wq_d = nc.declare_dram_parameter("wqT", [HID, JC], MM_DT, isOutput=False)
    wk_d = nc.declare_dram_parameter("wkT", [HID, JC], MM_DT, isOutput=False)
    wv_d = nc.declare_dram_parameter("wvT", [HID, JC], MM_DT, isOutput=False)
    wo_d = nc.declare_dram_parameter("woT", [JC, HID], MM_DT, isOutput=False)
    cos_d = nc.declare_dram_parameter("cosT2", [64, S], MM_DT, isOutput=False)
    sel2_d = nc.declare_dram_parameter("sel2", [128, 128], MM_DT, isOutput=False)
    sin_d = nc.declare_dram_parameter("sinT2", [64, S], MM_DT, isOutput=False)
    out_d = nc.declare_dram_parameter("out", [S, HID], F16, isOutput=True)

    rdram = nc.dram_tensor("rdram", [HPC, S], F32)

    with ExitStack() as ctx:
        tc = ctx.enter_context(tile.TileContext(nc, num_cores=NCORES))
        consts = ctx.enter_context(tc.tile_pool(name="consts", bufs=1))
        rt = ctx.enter_context(tc.tile_pool(name="rt", bufs=6))
        ptp = ctx.enter_context(tc.tile_pool(name="ptp", bufs=6))
        misc = ctx.enter_context(tc.tile_pool(name="misc", bufs=2))
        outp = ctx.enter_context(tc.tile_pool(name="outp", bufs=5))
        psum = ctx.enter_context(tc.tile_pool(name="psum", bufs=2, space="PSUM"))

        # ---- load constants, prioritized by first use ----
        hsT = consts.tile([128, NHC, S], MM_DT, tag="hsT")
        wsb = {}
        for name, d in (("wq", wq_d), ("wk", wk_d), ("wv", wv_d)):
            wsb[name] = consts.tile([128, NHC, JC], MM_DT, tag=name, name=f"w_{name}")
        for hc in range(NHC):
            nc.sync.dma_start(out=wsb["wq"][:, hc, :], in_=wq_d[hc * 128:(hc + 1) * 128, :])
            nc.sync.dma_start(out=hsT[:, hc, 0:512], in_=hsT_d[hc * 128:(hc + 1) * 128, 0:512])
        for hc in range(NHC):
            nc.sync.dma_start(out=wsb["wk"][:, hc, :], in_=wk_d[hc * 128:(hc + 1) * 128, :])
        cos2 = consts.tile([128, S], MM_DT, tag="cos2")
        sin2 = consts.tile([128, S], MM_DT, tag="sin2")
        nc.sync.dma_start(out=cos2[0:64, :], in_=cos_d[:])
        nc.sync.dma_start(out=sin2[0:64, :], in_=sin_d[:])
        nc.vector.tensor_copy(out=cos2[64:128, :], in_=cos2[0:64, :])
        nc.vector.tensor_copy(out=sin2[64:128, :], in_=sin2[0:64, :])
        for hc in range(NHC):
            nc.sync.dma_start(out=wsb["wv"][:, hc, :], in_=wv_d[hc * 128:(hc + 1) * 128, :])
        for w in range(1, 4):
            for hc in range(NHC):
                nc.sync.dma_start(
                    out=hsT[:, hc, 512 * w:512 * (w + 1)],
                    in_=hsT_d[hc * 128:(hc + 1) * 128, 512 * w:512 * (w + 1)],
                )
        wo = consts.tile([128, 4, HID], MM_DT, tag="wo")
        nc.sync.dma_start(out=wo[:], in_=wo_d[:].rearrange("(c p) j -> p c j", p=128))

        # ---- causal masks: full-width per-offset (chunk 0) + triangular block ----
        mask2 = [consts.tile([128, 2, 512], MM_DT, tag=f"mask{i}", name=f"mask{i}")
                 for i in range(4)]
        for i in range(4):
            nc.vector.memset(mask2[i][:], 1.0)
            for hl in range(2):
                nc.gpsimd.affine_select(
                    out=mask2[i][:, hl, :], in_=mask2[i][:, hl, :],
                    pattern=[[1, 512]], compare_op=Alu.is_ge,
                    fill=0.0, base=-128 * i, channel_multiplier=-1,
                )
        sel2 = consts.tile([128, 128], MM_DT, tag="sel2")
        nc.sync.dma_start(out=sel2[:], in_=sel2_d[:])
        tri2 = consts.tile([128, 2, 128], MM_DT, tag="tri2")
        nc.vector.memset(tri2[:], 1.0)
        for hl in range(2):
            nc.gpsimd.affine_select(
                out=tri2[:, hl, :], in_=tri2[:, hl, :],
                pattern=[[1, 128]], compare_op=Alu.is_ge,
                fill=0.0, base=0, channel_multiplier=-1,
            )

        qrope = [consts.tile([128, S], MM_DT, tag=f"qrope{i}", name=f"qrope{i}")
                 for i in range(4)]
        krope = [consts.tile([128, S], MM_DT, tag=f"krope{i}", name=f"krope{i}")
                 for i in range(4)]
        v_sb = consts.tile([128, NT, HPC, HD + 2], MM_DT, tag="v_sb")
        nc.vector.memset(v_sb[:, :, :, HD:HD + 2], 1.0)
        ctx_sb = [consts.tile([128, S], MM_DT, tag=f"ctx{i}", name=f"ctx_sb{i}")
                  for i in range(4)]

        # ---- filler machinery: fine-grained PE work queued into attention ----
        fillers = deque()

        def emit_fillers(n):
            for _ in range(n):
                if not fillers:
                    return
                fillers.popleft()()

        def drain_fillers():
            while fillers:
                fillers.popleft()()

        # ---- projection chains (emitted via fillers for waves >= 1) ----
        def qk_chain_items(sc, wname, hp):
            ssl = slice(sc * 512, (sc + 1) * 512)
            jcol = hp * 128
            dest = qrope if wname == "wq" else krope
            state = {}

            def mk_mm(hc):
                def f():
                    if hc == 0:
                        state["ps"] = psum.tile([128, 512], F32, tag="mm",
                                                name=f"qkps_{wname}{hp}_{sc}")
                    nc.tensor.matmul(
                        out=state["ps"][:],
                        lhsT=wsb[wname][:, hc, jcol:jcol + 128],
                        rhs=hsT[:, hc, ssl],
                        start=(hc == 0), stop=(hc == NHC - 1),
                    )
                return f

            def fin():
                raw_sb = misc.tile([128, 512], MM_DT, tag="qraw", bufs=4,
                                   name=f"raw_{wname}{hp}_{sc}")
                nc.scalar.copy(out=raw_sb[:], in_=state["ps"][:])
                rot_sb = misc.tile([128, 512], MM_DT, tag="qrot", bufs=4,
                                   name=f"rot_{wname}{hp}_{sc}")
                for hl in range(2):
                    b0 = 64 * hl
                    nc.scalar.dma_start(out=rot_sb[b0:b0 + 32, :],
                                        in_=raw_sb[b0 + 1:b0 + 64:2, :])
                    nc.scalar.dma_start(out=rot_sb[b0 + 32:b0 + 64, :],
                                        in_=raw_sb[b0:b0 + 63:2, :])
                t1 = rt.tile([128, 512], MM_DT, tag="rt")
                t2 = rt.tile([128, 512], MM_DT, tag="rt")
                nc.vector.tensor_tensor(out=t1[:], in0=raw_sb[:], in1=cos2[:, ssl], op=Alu.mult)
                nc.vector.tensor_tensor(out=t2[:], in0=rot_sb[:], in1=sin2[:, ssl], op=Alu.mult)
                nc.vector.tensor_add(out=dest[hp][:, ssl], in0=t1[:], in1=t2[:])

            return [mk_mm(hc) for hc in range(NHC)] + [fin]

        def v_chain_items(st):
            state = {}

            def mk_mm(hc):
                def f():
                    if hc == 0:
                        state["ps"] = psum.tile([128, JC], F32, tag="mm",
                                                name=f"vps_{st}")
                    nc.tensor.matmul(
                        out=state["ps"][:],
                        lhsT=hsT[:, hc, st * 128:(st + 1) * 128],
                        rhs=wsb["wv"][:, hc, :],
                        start=(hc == 0), stop=(hc == NHC - 1),
                    )
                return f

            def fin():
                nc.vector.tensor_copy(
                    out=v_sb[:, st, :, 0:HD],
                    in_=state["ps"][:].rearrange("p (h d) -> p h d", h=HPC),
                )

            return [mk_mm(hc) for hc in range(NHC)] + [fin]

        def proj_wave_items(sc):
            items = []
            for wname in ("wq", "wk"):
                for hp in range(4):
                    items.extend(qk_chain_items(sc, wname, hp))
            for st in range(4 * sc, 4 * sc + 4):
                items.extend(v_chain_items(st))
            return items

        # ---- o_proj chains for chunk j (queued as fillers into attn(j+1)) ----
        def oproj_chain_items(st, jc2):
            ssl2 = slice(st * 128, (st + 1) * 128)
            osl = slice(jc2 * 512, (jc2 + 1) * 512)
            state = {}

            def mk_mm(kc):
                def f():
                    if kc == 0:
                        state["ps"] = psum.tile([128, 512], F32, tag="mm",
                                                name=f"ops_{st}_{jc2}")
                    nc.tensor.matmul(
                        out=state["ps"][:],
                        lhsT=ctx_sb[kc][:, ssl2],
                        rhs=wo[:, kc, osl],
                        start=(kc == 0), stop=(kc == 3),
                    )
                return f

            def fin():
                o_sb = outp.tile([128, 512], F16, tag="osb")
                nc.vector.tensor_copy(out=o_sb[:], in_=state["ps"][:])
                nc.sync.dma_start(out=out_d[ssl2, osl], in_=o_sb[:])

            return [mk_mm(kc) for kc in range(4)] + [fin]

        bc_tiles = {}

        def norm_one(j, hp):
            qsl = slice(j * 512, (j + 1) * 512)
            bc = bc_tiles[j][hp]
            nc.vector.reciprocal_approx_fast(out=bc[:], in_=bc[:])
            nc.vector.tensor_tensor(
                out=ctx_sb[hp][:, qsl], in0=ctx_sb[hp][:, qsl], in1=bc[:], op=Alu.mult,
            )

        def norm_block(j):
            for hp in range(4):
                norm_one(j, hp)

        def oproj_items(j):
            items = []
            for st in range(4 * j, 4 * j + 4):
                for jc2 in range(2):
                    items.extend(oproj_chain_items(st, jc2))
            return items

        # ---- attention chunk j: skew-2 pipeline with fillers ----
        def attn_block(j):
            qsl = slice(j * 512, (j + 1) * 512)
            nt = 4 * j + 4
            if j >= 1:
                # diagonal tiles first (i=0 is full width -> clean psum start),
                # then the full-width off-diagonal tiles; last one carries stop.
                order = [4 * j + i for i in range(4)] + list(range(4 * j))
            else:
                order = list(range(nt))
            for hp in range(4):
                ctx_ps = [psum.tile([HD + 2, 512], F32, tag="ctx", name=f"ctx_ps{_i}")
                          for _i in range(2)]
                pts = {}

                def s_part(slot):
                    t = order[slot]
                    diag_i = t - 4 * j
                    shrink = j >= 1 and diag_i >= 0
                    c0 = 128 * diag_i if shrink else 0
                    sc_ps = psum.tile([128, 2, 512], F32, tag="sc")
                    ksl = slice(t * 128, (t + 1) * 128)
                    for hl in range(2):
                        pr = slice(64 * hl, 64 * hl + 64)
                        nc.tensor.matmul(
                            out=sc_ps[:, hl, c0:512],
                            lhsT=krope[hp][pr, ksl],
                            rhs=qrope[hp][pr, 512 * j + c0:512 * (j + 1)],
                            start=True, stop=True,
                        )
                    pt = ptp.tile([128, 2, 512], MM_DT, tag="pt")
                    nc.scalar.activation(out=pt[:, :, c0:512], in_=sc_ps[:, :, c0:512],
                                         func=AF.Exp, scale=float(SCALE))
                    if diag_i >= 0:
                        if shrink:
                            # in-place triangular mask on the on-diagonal block
                            blk = slice(c0, c0 + 128)
                            nc.vector.tensor_tensor(
                                out=pt[:, :, blk], in0=pt[:, :, blk], in1=tri2[:],
                                op=Alu.mult,
                            )
                        else:
                            ptm = ptp.tile([128, 2, 512], MM_DT, tag="ptm", bufs=3)
                            nc.vector.tensor_tensor(
                                out=ptm[:], in0=pt[:], in1=mask2[diag_i][:], op=Alu.mult,
                            )
                            pt = ptm
                    pts[slot] = (pt, c0)

                def c_part(slot):
                    t = order[slot]
                    pt, c0 = pts.pop(slot)
                    for hl in range(2):
                        nc.tensor.matmul(
                            out=ctx_ps[hl][:, c0:512],
                            lhsT=v_sb[:, t, 2 * hp + hl, :],
                            rhs=pt[:, hl, c0:512],
                            start=(slot == 0), stop=(slot == nt - 1),
                        )

                for slot in range(nt + 2):
                    if slot < nt:
                        s_part(slot)
                    emit_fillers(1)
                    if slot >= 2:
                        c_part(slot - 2)
                    emit_fillers(1)
                    if slot == 6 and j == 3 and hp >= 1:
                        norm_one(3, hp - 1)

                last = j == 3 and hp == 3
                for hl in range(2):
                    pr = slice(64 * hl, 64 * hl + 64)
                    srow = misc.tile([128, 512], F32, tag="srow", bufs=4)
                    nc.vector.tensor_copy(out=srow[64:65, :], in_=ctx_ps[hl][HD:HD + 1, :])
                    nc.gpsimd.dma_start(
                        out=rdram[2 * hp + hl:2 * hp + hl + 1, qsl],
                        in_=srow[64:65, :],
                    )
                    if last:
                        nc.scalar.copy(out=ctx_sb[hp][pr, qsl], in_=ctx_ps[hl][0:HD, :])
                    else:
                        nc.vector.tensor_copy(out=ctx_sb[hp][pr, qsl], in_=ctx_ps[hl][0:HD, :])
                bc = misc.tile([128, 512], F32, tag="bc", bufs=5, name=f"bc{j}_{hp}")
                if last:
                    for q4 in range(2):
                        for hl in range(2):
                            p0 = 64 * hl + 32 * q4
                            nc.sync.dma_start(
                                out=bc[p0:p0 + 32, :],
                                in_=rdram[2 * hp + hl:2 * hp + hl + 1, qsl].partition_broadcast(32),
                            )
                else:
                    nc.sync.dma_start(
                        out=bc[0:64, :],
                        in_=rdram[2 * hp:2 * hp + 1, qsl].partition_broadcast(64),
                    )
                    nc.sync.dma_start(
                        out=bc[64:128, :],
                        in_=rdram[2 * hp + 1:2 * hp + 2, qsl].partition_broadcast(64),
                    )
                bc_tiles.setdefault(j, []).append(bc)
                if j == 3 and hp == 3:
                    norm_one(3, 3)

        # ---- schedule ----
        # wave 0 projections as a straight block (DMA-limited startup)
        for it in proj_wave_items(0):
            it()

        fillers.extend(proj_wave_items(1))
        attn_block(0)
        drain_fillers()

        norm_block(0)
        fillers.extend(proj_wave_items(2))
        attn_block(1)
        drain_fillers()

        norm_block(1)
        fillers.extend(proj_wave_items(3))
        fillers.extend(oproj_items(0))
        attn_block(2)
        drain_fillers()

        norm_block(2)
        fillers.extend(oproj_items(1))
        fillers.extend(oproj_items(2))
        attn_block(3)
        drain_fillers()

        for it in oproj_items(3):
            it()

    nc.finalize()
    return nc


def _rope_tables():
    inv_freq = (1.0 / (ROPE_BASE ** (np.arange(0, HD, 2, dtype=np.float32) / np.float32(HD)))).astype(np.float32)
    t = np.arange(S, dtype=np.float32)
    freqs = np.outer(t, inv_freq).astype(np.float32)          # [S, 32]
    emb = np.concatenate([freqs, freqs], axis=-1)             # [S, 64]
    return np.cos(emb).astype(np.float32), np.sin(emb).astype(np.float32)


def _rot_weights(W):
    """Rows of Wr give rotated(x) = cat(-x2, x1) of x = W @ h per 64-dim head."""
    Wr = np.empty_like(W)
    for h in range(NH):
        b = h * HD
        Wr[b:b + 32] = -W[b + 1:b + HD:2]
        Wr[b + 32:b + HD] = W[b:b + HD:2]
    return Wr


def prepare_in_maps(hidden_states, Wq, Wk, Wv, Wo):
    sel2 = np.zeros((128, 128), dtype=np.float32)
    sel2[64, 0:64] = 1.0
    sel2[65, 64:128] = 1.0
    cos, sin = _rope_tables()                                  # [S, 64]
    cos2 = np.ascontiguousarray(cos.T)                         # [64, S]
    sin2 = np.ascontiguousarray(sin.T)
    # sign of the rotation (-x2 for d<32) folded into the sin table
    sin2[0:32] *= -1.0
    if MM_DT == F16:
        f16 = np.float16
    else:
        import ml_dtypes
        f16 = ml_dtypes.bfloat16
    in_maps = []
    for c in range(NCORES):
        b, hg = c // 2, c % 2
        sl = slice(JC * hg, JC * (hg + 1))
        m = {
            "hsT": np.ascontiguousarray(hidden_states[b].T).astype(f16),
            "wqT": np.ascontiguousarray(Wq[sl].T).astype(f16),
            "wkT": np.ascontiguousarray(Wk[sl].T).astype(f16),
            "wvT": np.ascontiguousarray(Wv[sl].T).astype(f16),
            "woT": np.ascontiguousarray(Wo[:, sl].T).astype(f16),
            "sel2": sel2.astype(f16),
            "cosT2": cos2.astype(f16),
            "sinT2": sin2.astype(f16),
        }
        in_maps.append(m)
    return in_maps


def run(inputs, trace=False, tmpdir=None):
    global _PROGRAM
    if _PROGRAM is None:
        _PROGRAM = build()
    nc = _PROGRAM
    in_maps = prepare_in_maps(
        np.asarray(inputs["hidden_states"], dtype=np.float32),
        np.asarray(inputs["Wq"], dtype=np.float32),
        np.asarray(inputs["Wk"], dtype=np.float32),
        np.asarray(inputs["Wv"], dtype=np.float32),
        np.asarray(inputs["Wo"], dtype=np.float32),
    )
    res = run_bass_kernel_spmd(nc, in_maps, list(range(NCORES)), trace=trace, tmpdir=tmpdir)
    out = np.empty((B, S, HID), dtype=np.float32)
    for b in range(B):
        lo = res.results[2 * b]["out"].astype(np.float32)
        hi = res.results[2 * b + 1]["out"].astype(np.float32)
        out[b] = lo + hi
    return out, res


def kernel(**inputs):
    out, _ = run(inputs)
    return out


# revision 33
# speedup vs baseline: 1.0226x; 1.0028x over previous
"""Causal multi-head attention (B=4, S=2048, HID=1024, 16 heads x 64) with RoPE
on 8 TRN2 NeuronCores.

Sharding: core c -> batch b = c//2, head-group hg = c%2 (8 heads each).

Schedule: projections are split into 4 column-waves; wave j+1's matmul chains
(and the previous chunk's o_proj chains) are emitted as fine-grained FILLER
instructions inside attention chunk j's inner loop, with the score->exp->ctx
pipeline software-skewed by 2 tiles. This keeps the Tensor engine queue free
of head-of-line stalls on the Scalar (exp) engine, holding the PE busy so it
stays in its fast p-state.

Diagonal score tiles only compute/exp/accumulate the causally visible column
range; the on-diagonal 128-col block is masked in-place with one triangular
0/1 multiply on Vector (q-chunk 0 keeps the simpler full-width masked path).

Outputs are fp16 partial o_proj results DMA'd straight to DRAM; the pairwise
head-group reduction happens on the host during unsharding (no collective).

All matmuls run in fp16 (fp32 PSUM accumulation).
"""
import os as _os
import numpy as np
from collections import deque
from contextlib import ExitStack

import concourse.bass as bass
import concourse.tile as tile
import concourse.mybir as mybir
from concourse import bacc
from concourse.alu_op_type import AluOpType
from concourse.bass_utils import run_bass_kernel_spmd

F32 = mybir.dt.float32
F16 = mybir.dt.float16
BF16 = mybir.dt.bfloat16
MM_DT = BF16 if _os.environ.get("KMM", "f16") == "bf16" else F16
AF = mybir.ActivationFunctionType
Alu = AluOpType

B, S, HID = 4, 2048, 1024
NH, HD = 16, 64
SCALE = 1.0 / np.sqrt(HD)
ROPE_BASE = 10000.0
NCORES = 8
HPC = 8          # heads per core
JC = 512         # head dims per core
NJ = 4           # q chunks of 512
NT = 16          # kk tiles of 128
NSC = 4          # s chunks of 512 for projections
NHC = 8          # hid chunks of 128 (contraction)

_PROGRAM = None


def build():
    nc = bacc.Bacc("TRN2", target_bir_lowering=False, debug=False)

    hsT_d = nc.declare_dram_parameter("hsT", [HID, S], MM_DT, isOutput=False)
    wq_d = nc.declare_dram_parameter("wqT", [HID, JC], MM_DT, isOutput=False)
    wk_d = nc.declare_dram_parameter("wkT", [HID, JC], MM_DT, isOutput=False)
    wv_d = nc.declare_dram_parameter("wvT", [HID, JC], MM_DT, isOutput=False)
    wo_d = nc.declare_dram_parameter("woT", [JC, HID], MM_DT, isOutput=False)
    cos_d = nc.declare_dram_parameter("cosT2", [64, S], MM_DT, isOutput=False)
    sel2_d = nc.declare_dram_parameter("sel2", [128, 128], MM_DT, isOutput=False)
    sin_d = nc.declare_dram_parameter("sinT2", [64, S], MM_DT, isOutput=False)
    out_d = nc.declare_dram_parameter("out", [S, HID], F16, isOutput=True)

    rdram = nc.dram_tensor("rdram", [HPC, S], F32)

    with ExitStack() as ctx:
        tc = ctx.enter_context(tile.TileContext(nc, num_cores=NCORES))
        consts = ctx.enter_context(tc.tile_pool(name="consts", bufs=1))
        rt = ctx.enter_context(tc.tile_pool(name="rt", bufs=6))
        ptp = ctx.enter_context(tc.tile_pool(name="ptp", bufs=6))
        misc = ctx.enter_context(tc.tile_pool(name="misc", bufs=2))
        outp = ctx.enter_context(tc.tile_pool(name="outp", bufs=5))
        psum = ctx.enter_context(tc.tile_pool(name="psum", bufs=2, space="PSUM"))

        # ---- load constants, prioritized by first use ----
        hsT = consts.tile([128, NHC, S], MM_DT, tag="hsT")
        wsb = {}
        for name, d in (("wq", wq_d), ("wk", wk_d), ("wv", wv_d)):
            wsb[name] = consts.tile([128, NHC, JC], MM_DT, tag=name, name=f"w_{name}")
        for hc in range(NHC):
            nc.sync.dma_start(out=wsb["wq"][:, hc, :], in_=wq_d[hc * 128:(hc + 1) * 128, :])
            nc.sync.dma_start(out=hsT[:, hc, 0:512], in_=hsT_d[hc * 128:(hc + 1) * 128, 0:512])
        for hc in range(NHC):
            nc.sync.dma_start(out=wsb["wk"][:, hc, :], in_=wk_d[hc * 128:(hc + 1) * 128, :])
        cos2 = consts.tile([128, S], MM_DT, tag="cos2")
        sin2 = consts.tile([128, S], MM_DT, tag="sin2")
        nc.sync.dma_start(out=cos2[0:64, :], in_=cos_d[:])
        nc.sync.dma_start(out=sin2[0:64, :], in_=sin_d[:])
        nc.vector.tensor_copy(out=cos2[64:128, :], in_=cos2[0:64, :])
        nc.vector.tensor_copy(out=sin2[64:128, :], in_=sin2[0:64, :])
        for hc in range(NHC):
            nc.sync.dma_start(out=wsb["wv"][:, hc, :], in_=wv_d[hc * 128:(hc + 1) * 128, :])
        for w in range(1, 4):
            for hc in range(NHC):
                nc.sync.dma_start(
                    out=hsT[:, hc, 512 * w:512 * (w + 1)],
                    in_=hsT_d[hc * 128:(hc + 1) * 128, 512 * w:512 * (w + 1)],
                )
        wo = consts.tile([128, 4, HID], MM_DT, tag="wo")
        nc.sync.dma_start(out=wo[:], in_=wo_d[:].rearrange("(c p) j -> p c j", p=128))

        # ---- causal masks: full-width per-offset (chunk 0) + triangular block ----
        mask2 = [consts.tile([128, 2, 512], MM_DT, tag=f"mask{i}", name=f"mask{i}")
                 for i in range(4)]
        for i in range(4):
            nc.vector.memset(mask2[i][:], 1.0)
            for hl in range(2):
                nc.gpsimd.affine_select(
                    out=mask2[i][:, hl, :], in_=mask2[i][:, hl, :],
                    pattern=[[1, 512]], compare_op=Alu.is_ge,
                    fill=0.0, base=-128 * i, channel_multiplier=-1,
                )
        sel2 = consts.tile([128, 128], MM_DT, tag="sel2")
        nc.sync.dma_start(out=sel2[:], in_=sel2_d[:])
        tri2 = consts.tile([128, 2, 128], MM_DT, tag="tri2")
        nc.vector.memset(tri2[:], 1.0)
        for hl in range(2):
            nc.gpsimd.affine_select(
                out=tri2[:, hl, :], in_=tri2[:, hl, :],
                pattern=[[1, 128]], compare_op=Alu.is_ge,
                fill=0.0, base=0, channel_multiplier=-1,
            )

        qrope = [consts.tile([128, S], MM_DT, tag=f"qrope{i}", name=f"qrope{i}")
                 for i in range(4)]
        krope = [consts.tile([128, S], MM_DT, tag=f"krope{i}", name=f"krope{i}")
                 for i in range(4)]
        v_sb = consts.tile([128, NT, HPC, HD + 2], MM_DT, tag="v_sb")
        nc.vector.memset(v_sb[:, :, :, HD:HD + 2], 1.0)
        ctx_sb = [consts.tile([128, S], MM_DT, tag=f"ctx{i}", name=f"ctx_sb{i}")
                  for i in range(4)]

        # ---- filler machinery: fine-grained PE work queued into attention ----
        fillers = deque()

        def emit_fillers(n):
            for _ in range(n):
                if not fillers:
                    return
                fillers.popleft()()

        def drain_fillers():
            while fillers:
                fillers.popleft()()

        # ---- projection chains (emitted via fillers for waves >= 1) ----
        def qk_chain_items(sc, wname, hp):
            ssl = slice(sc * 512, (sc + 1) * 512)
            jcol = hp * 128
            dest = qrope if wname == "wq" else krope
            state = {}

            def mk_mm(hc):
                def f():
                    if hc == 0:
                        state["ps"] = psum.tile([128, 512], F32, tag="mm",
                                                name=f"qkps_{wname}{hp}_{sc}")
                    nc.tensor.matmul(
                        out=state["ps"][:],
                        lhsT=wsb[wname][:, hc, jcol:jcol + 128],
                        rhs=hsT[:, hc, ssl],
                        start=(hc == 0), stop=(hc == NHC - 1),
                    )
                return f

            def fin():
                raw_sb = misc.tile([128, 512], MM_DT, tag="qraw", bufs=4,
                                   name=f"raw_{wname}{hp}_{sc}")
                nc.scalar.copy(out=raw_sb[:], in_=state["ps"][:])
                rot_sb = misc.tile([128, 512], MM_DT, tag="qrot", bufs=4,
                                   name=f"rot_{wname}{hp}_{sc}")
                for hl in range(2):
                    b0 = 64 * hl
                    nc.scalar.dma_start(out=rot_sb[b0:b0 + 32, :],
                                        in_=raw_sb[b0 + 1:b0 + 64:2, :])
                    nc.scalar.dma_start(out=rot_sb[b0 + 32:b0 + 64, :],
                                        in_=raw_sb[b0:b0 + 63:2, :])
                t1 = rt.tile([128, 512], MM_DT, tag="rt")
                t2 = rt.tile([128, 512], MM_DT, tag="rt")
                nc.vector.tensor_tensor(out=t1[:], in0=raw_sb[:], in1=cos2[:, ssl], op=Alu.mult)
                nc.vector.tensor_tensor(out=t2[:], in0=rot_sb[:], in1=sin2[:, ssl], op=Alu.mult)
                nc.vector.tensor_add(out=dest[hp][:, ssl], in0=t1[:], in1=t2[:])

            return [mk_mm(hc) for hc in range(NHC)] + [fin]

        def v_chain_items(st):
            state = {}

            def mk_mm(hc):
                def f():
                    if hc == 0:
                        state["ps"] = psum.tile([128, JC], F32, tag="mm",
                                                name=f"vps_{st}")
                    nc.tensor.matmul(
                        out=state["ps"][:],
                        lhsT=hsT[:, hc, st * 128:(st + 1) * 128],
                        rhs=wsb["wv"][:, hc, :],
                        start=(hc == 0), stop=(hc == NHC - 1),
                    )
                return f

            def fin():
                nc.vector.tensor_copy(
                    out=v_sb[:, st, :, 0:HD],
                    in_=state["ps"][:].rearrange("p (h d) -> p h d", h=HPC),
                )

            return [mk_mm(hc) for hc in range(NHC)] + [fin]

        def proj_wave_items(sc):
            items = []
            for wname in ("wq", "wk"):
                for hp in range(4):
                    items.extend(qk_chain_items(sc, wname, hp))
            for st in range(4 * sc, 4 * sc + 4):
                items.extend(v_chain_items(st))
            return items

        # ---- o_proj chains for chunk j (queued as fillers into attn(j+1)) ----
        def oproj_chain_items(st, jc2):
            ssl2 = slice(st * 128, (st + 1) * 128)
            osl = slice(jc2 * 512, (jc2 + 1) * 512)
            state = {}

            def mk_mm(kc):
                def f():
                    if kc == 0:
                        state["ps"] = psum.tile([128, 512], F32, tag="mm",
                                                name=f"ops_{st}_{jc2}")
                    nc.tensor.matmul(
                        out=state["ps"][:],
                        lhsT=ctx_sb[kc][:, ssl2],
                        rhs=wo[:, kc, osl],
                        start=(kc == 0), stop=(kc == 3),
                    )
                return f

            def fin():
                o_sb = outp.tile([128, 512], F16, tag="osb")
                nc.vector.tensor_copy(out=o_sb[:], in_=state["ps"][:])
                nc.sync.dma_start(out=out_d[ssl2, osl], in_=o_sb[:])

            return [mk_mm(kc) for kc in range(4)] + [fin]

        bc_tiles = {}

        def norm_one(j, hp):
            qsl = slice(j * 512, (j + 1) * 512)
            bc = bc_tiles[j][hp]
            nc.vector.reciprocal_approx_fast(out=bc[:], in_=bc[:])
            nc.vector.tensor_tensor(
                out=ctx_sb[hp][:, qsl], in0=ctx_sb[hp][:, qsl], in1=bc[:], op=Alu.mult,
            )

        def norm_block(j):
            for hp in range(4):
                norm_one(j, hp)

        def oproj_items(j):
            items = []
            for st in range(4 * j, 4 * j + 4):
                for jc2 in range(2):
                    items.extend(oproj_chain_items(st, jc2))
            return items

        # ---- attention chunk j: skew-2 pipeline with fillers ----
        def attn_block(j):
            qsl = slice(j * 512, (j + 1) * 512)
            nt = 4 * j + 4
            if j >= 1:
                # diagonal tiles first (i=0 is full width -> clean psum start),
                # then the full-width off-diagonal tiles; last one carries stop.
                order = [4 * j + i for i in range(4)] + list(range(4 * j))
            else:
                order = list(range(nt))
            for hp in range(4):
                ctx_ps = [psum.tile([HD + 2, 512], F32, tag="ctx", name=f"ctx_ps{_i}")
                          for _i in range(2)]
                pts = {}

                def s_part(slot):
                    t = order[slot]
                    diag_i = t - 4 * j
                    shrink = j >= 1 and diag_i >= 0
                    c0 = 128 * diag_i if shrink else 0
                    sc_ps = psum.tile([128, 2, 512], F32, tag="sc")
                    ksl = slice(t * 128, (t + 1) * 128)
                    for hl in range(2):
                        pr = slice(64 * hl, 64 * hl + 64)
                        nc.tensor.matmul(
                            out=sc_ps[:, hl, c0:512],
                            lhsT=krope[hp][pr, ksl],
                            rhs=qrope[hp][pr, 512 * j + c0:512 * (j + 1)],
                            start=True, stop=True,
                        )
                    pt = ptp.tile([128, 2, 512], MM_DT, tag="pt")
                    nc.scalar.activation(out=pt[:, :, c0:512], in_=sc_ps[:, :, c0:512],
                                         func=AF.Exp, scale=float(SCALE))
                    if diag_i >= 0:
                        if shrink:
                            # in-place triangular mask on the on-diagonal block
                            blk = slice(c0, c0 + 128)
                            nc.vector.tensor_tensor(
                                out=pt[:, :, blk], in0=pt[:, :, blk], in1=tri2[:],
                                op=Alu.mult,
                            )
                        else:
                            ptm = ptp.tile([128, 2, 512], MM_DT, tag="ptm", bufs=3)
                            nc.vector.tensor_tensor(
                                out=ptm[:], in0=pt[:], in1=mask2[diag_i][:], op=Alu.mult,
                            )
                            pt = ptm
                    pts[slot] = (pt, c0)

                def c_part(slot):
                    t = order[slot]
                    pt, c0 = pts.pop(slot)
                    for hl in range(2):
                        nc.tensor.matmul(
                            out=ctx_ps[hl][:, c0:512],
                            lhsT=v_sb[:, t, 2 * hp + hl, :],
                            rhs=pt[:, hl, c0:512],
                            start=(slot == 0), stop=(slot == nt - 1),
                        )

                for slot in range(nt + 2):
                    if slot < nt:
                        s_part(slot)
                    emit_fillers(1)
                    if slot >= 2:
                        c_part(slot - 2)
                    emit_fillers(1)
                    if slot == 6 and j == 3 and hp >= 1:
                        norm_one(3, hp - 1)

                last = j == 3 and hp == 3
                for hl in range(2):
                    pr = slice(64 * hl, 64 * hl + 64)
                    srow = misc.tile([128, 512], F32, tag="srow", bufs=4)
                    nc.vector.tensor_copy(out=srow[64:65, :], in_=ctx_ps[hl][HD:HD + 1, :])
                    nc.gpsimd.dma_start(
                        out=rdram[2 * hp + hl:2 * hp + hl + 1, qsl],
                        in_=srow[64:65, :],
                    )
                    if last:
                        nc.scalar.copy(out=ctx_sb[hp][pr, qsl], in_=ctx_ps[hl][0:HD, :])
                    else:
                        nc.vector.tensor_copy(out=ctx_sb[hp][pr, qsl], in_=ctx_ps[hl][0:HD, :])
                bc = misc.tile([128, 512], F32, tag="bc", bufs=5, name=f"bc{j}_{hp}")
                if last:
                    for q4 in range(2):
                        for hl in range(2):
                            p0 = 64 * hl + 32 * q4
                            nc.sync.dma_start(
                                out=bc[p0:p0 + 32, :],
                                in_=rdram[2 * hp + hl:2 * hp + hl + 1, qsl].partition_broadcast(32),
                            )
                else:
                    nc.sync.dma_start(
                        out=bc[0:64, :],
                        in_=rdram[2 * hp:2 * hp + 1, qsl].partition_broadcast(64),
                    )
                    nc.sync.dma_start(
                        out=bc[64:128, :],
                        in_=rdram[2 * hp + 1:2 * hp + 2, qsl].partition_broadcast(64),
                    )
                bc_tiles.setdefault(j, []).append(bc)

        # ---- schedule ----
        # wave 0 projections as a straight block (DMA-limited startup)
        for it in proj_wave_items(0):
            it()

        fillers.extend(proj_wave_items(1))
        attn_block(0)
        drain_fillers()

        norm_block(0)
        fillers.extend(proj_wave_items(2))
        attn_block(1)
        drain_fillers()

        norm_block(1)
        fillers.extend(proj_wave_items(3))
        fillers.extend(oproj_items(0))
        attn_block(2)
        drain_fillers()

        norm_block(2)
        fillers.extend(oproj_items(1))
        fillers.extend(oproj_items(2))
        attn_block(3)
        drain_fillers()

        chains3 = [oproj_chain_items(st, jc2)
                   for st in range(12, 16) for jc2 in range(2)]
        for it in chains3[0][0:3]:
            it()
        for it in chains3[1][0:3]:
            it()
        norm_one(3, 3)
        for it in chains3[0][3:]:
            it()
        for it in chains3[1][3:]:
            it()
        for ch in chains3[2:]:
            for it in ch:
                it()

    nc.finalize()
    return nc


def _rope_tables():
    inv_freq = (1.0 / (ROPE_BASE ** (np.arange(0, HD, 2, dtype=np.float32) / np.float32(HD)))).astype(np.float32)
    t = np.arange(S, dtype=np.float32)
    freqs = np.outer(t, inv_freq).astype(np.float32)          # [S, 32]
    emb = np.concatenate([freqs, freqs], axis=-1)             # [S, 64]
    return np.cos(emb).astype(np.float32), np.sin(emb).astype(np.float32)


def _rot_weights(W):
    """Rows of Wr give rotated(x) = cat(-x2, x1) of x = W @ h per 64-dim head."""
    Wr = np.empty_like(W)
    for h in range(NH):
        b = h * HD
        Wr[b:b + 32] = -W[b + 1:b + HD:2]
        Wr[b + 32:b + HD] = W[b:b + HD:2]
    return Wr


def prepare_in_maps(hidden_states, Wq, Wk, Wv, Wo):
    sel2 = np.zeros((128, 128), dtype=np.float32)
    sel2[64, 0:64] = 1.0
    sel2[65, 64:128] = 1.0
    cos, sin = _rope_tables()                                  # [S, 64]
    cos2 = np.ascontiguousarray(cos.T)                         # [64, S]
    sin2 = np.ascontiguousarray(sin.T)
    # sign of the rotation (-x2 for d<32) folded into the sin table
    sin2[0:32] *= -1.0
    if MM_DT == F16:
        f16 = np.float16
    else:
        import ml_dtypes
        f16 = ml_dtypes.bfloat16
    in_maps = []
    for c in range(NCORES):
        b, hg = c // 2, c % 2
        sl = slice(JC * hg, JC * (hg + 1))
        m = {
            "hsT": np.ascontiguousarray(hidden_states[b].T).astype(f16),
            "wqT": np.ascontiguousarray(Wq[sl].T).astype(f16),
            "wkT": np.ascontiguousarray(Wk[sl].T).astype(f16),
            "wvT": np.ascontiguousarray(Wv[sl].T).astype(f16),
            "woT": np.ascontiguousarray(Wo[:, sl].T).astype(f16),
            "sel2": sel2.astype(f16),
            "cosT2": cos2.astype(f16),
            "sinT2": sin2.astype(f16),
        }
        in_maps.append(m)
    return in_maps


def run(inputs, trace=False, tmpdir=None):
    global _PROGRAM
    if _PROGRAM is None:
        _PROGRAM = build()
    nc = _PROGRAM
    in_maps = prepare_in_maps(
        np.asarray(inputs["hidden_states"], dtype=np.float32),
        np.asarray(inputs["Wq"], dtype=np.float32),
        np.asarray(inputs["Wk"], dtype=np.float32),
        np.asarray(inputs["Wv"], dtype=np.float32),
        np.asarray(inputs["Wo"], dtype=np.float32),
    )
    res = run_bass_kernel_spmd(nc, in_maps, list(range(NCORES)), trace=trace, tmpdir=tmpdir)
    out = np.empty((B, S, HID), dtype=np.float32)
    for b in range(B):
        lo = res.results[2 * b]["out"].astype(np.float32)
        hi = res.results[2 * b + 1]["out"].astype(np.float32)
        out[b] = lo + hi
    return out, res


def kernel(**inputs):
    out, _ = run(inputs)
    return out
